# revision 8
# baseline (speedup 1.0000x reference)
"""CosFace loss (B=1024, D=512, C=100000) on 8 Trainium2 NeuronCores.

Strategy (tensor-parallel classification head, per sharding hint):
  - Classes sharded 12500/core (padded to 12544 = 98*128 with zero rows).
  - Host prep: weight rows L2-normalized, shard transposed to [D, C_loc]
    (matmul needs the contraction dim D on partitions), x transposed,
    weight[label] rows gathered.
  - Device per core: stream wT chunks, f32r matmuls accumulate cosine
    tiles [128b, 2048c] in PSUM; fused ScalarE pass computes
    exp(S*rsqrt(|x_b|^2)*cos - 64) with a per-partition scale vector and
    accum_out giving per-row partial sum-of-exp (the fixed shift 64 = S
    bounds |logits|, so no running max is needed: exp args are in
    [-128, 0], safely inside fp32 range).
  - One 4KB AllReduce of the [1024] partial sums.
  - Margin fixup: the label logit must be S*(cos-M), not S*cos.  Each
    core computes cos_label from x and weight[label] directly, swaps the
    two exp terms in the reduced sum, then logz = 64 + ln(sum),
    loss = mean(logz - (S*cos_label - S*M)).  Core 0's output is used.
"""

import numpy as np

import concourse.bass as bass
import concourse.mybir as mybir
import concourse.tile as tile
from concourse import bacc
from concourse.bass_utils import run_bass_kernel_spmd

B, D, C = 1024, 512, 100000
S, MARGIN = 64.0, 0.35
SHIFT = 64.0
NCORES = 8
CSHARD = C // NCORES          # 12500 real classes per core
CLOC = 12544                  # padded (98 * 128)
KT = D // 128                 # 4 contraction tiles
MT = B // 128                 # 8 batch tiles
CHUNK = 2048                  # classes per PSUM group (4 banks)
NCHUNK = (CLOC + CHUNK - 1) // CHUNK  # 7 (6 full + 256)

F32 = mybir.dt.float32
F32R = mybir.dt.float32r
AF = mybir.ActivationFunctionType
AX = mybir.AxisListType
ALU = mybir.AluOpType

_NC = None
LAST_RESULTS = None
DEBUG_DUMPS = False


def _body(nc, tc, xt, xn, wt, wl, loss, dbg=None):
    from contextlib import ExitStack
    with ExitStack() as ctx:
        singles = ctx.enter_context(tc.tile_pool(name="singles", bufs=1))
        wpool = ctx.enter_context(tc.tile_pool(name="wpool", bufs=2))
        scrp = ctx.enter_context(tc.tile_pool(name="scrp", bufs=2))
        psump = ctx.enter_context(tc.tile_pool(name="psump", bufs=2, space="PSUM"))
        dram = ctx.enter_context(tc.tile_pool(name="dram", bufs=1, space="DRAM"))
        # ---- resident inputs ----
        xt_sb = singles.tile([128, KT, B], F32R)
        nc.sync.dma_start(out=xt_sb[:, :, :],
                          in_=xt.ap().rearrange("(k p) b -> p k b", p=128))
        xn_sb = singles.tile([128, MT, D], F32)
        nc.sync.dma_start(out=xn_sb[:, :, :],
                          in_=xn.ap().rearrange("(m p) d -> p m d", p=128))
        wl_sb = singles.tile([128, MT, D], F32)
        nc.sync.dma_start(out=wl_sb[:, :, :],
                          in_=wl.ap().rearrange("(m p) d -> p m d", p=128))

        # const bias columns (activation bias must be a [P,1] AP)
        def const_col(val):
            t = singles.tile([128, 1], F32)
            nc.vector.memset(t[:, :], val)
            return t

        cb_m64 = const_col(-SHIFT)
        cb_m86 = const_col(-(SHIFT + S * MARGIN))
        cb_p86 = const_col(SHIFT + S * MARGIN - 66.0 * float(np.log(2.0)))

        # ---- x row norms -> per-partition exp scales ----
        nsq = singles.tile([128, MT], F32)
        junk = singles.tile([128, D], F32)
        for m in range(MT):
            nc.scalar.activation(junk[:, :], xn_sb[:, m, :], AF.Square,
                                 accum_out=nsq[:, m:m + 1])
        # rsqrt via exp(-0.5*ln(.)): keeps ScalarE on one table set (exp+ln)
        lt = singles.tile([128, MT], F32)
        nc.scalar.activation(lt[:, :], nsq[:, :], AF.Ln)
        rx_raw = singles.tile([128, MT], F32)
        nc.scalar.activation(rx_raw[:, :], lt[:, :], AF.Exp, scale=-0.5)
        rx_s = singles.tile([128, MT], F32)
        nc.vector.tensor_scalar_mul(rx_s[:, :], rx_raw[:, :], S)

        # ---- main loop: cosine matmuls + fused exp/accumulate ----
        sums = singles.tile([128, MT, NCHUNK], F32)
        nc.vector.memset(sums[:, :, :], 0.0)
        wt_v = wt.ap().rearrange("(k p) c -> p k c", p=128)
        for c in range(NCHUNK):
            c0 = c * CHUNK
            ncls = min(CHUNK, CLOC - c0)
            wt_c = wpool.tile([128, KT, CHUNK], F32R, tag="wt")
            nc.sync.dma_start(out=wt_c[:, :, :ncls], in_=wt_v[:, :, c0:c0 + ncls])
            for m in range(MT):
                g = psump.tile([128, CHUNK], F32, tag="g")
                for k in range(KT):
                    lhsT = xt_sb[:, k, m * 128:(m + 1) * 128]
                    for n in range(0, ncls, 512):
                        nsz = min(512, ncls - n)
                        nc.tensor.matmul(g[:, n:n + nsz], lhsT,
                                         wt_c[:, k, n:n + nsz],
                                         start=(k == 0), stop=(k == KT - 1))
                scr = scrp.tile([128, CHUNK], F32, tag="scr")
                nc.scalar.activation(scr[:, :ncls], g[:, :ncls], AF.Exp,
                                     bias=cb_m64[:, :], scale=rx_s[:, m:m + 1],
                                     accum_out=sums[:, m, c:c + 1])

        # ---- label fixup inputs (independent of main loop; overlaps) ----
        prod = singles.tile([128, MT, D], F32)
        nc.vector.tensor_mul(prod[:, :, :], xn_sb[:, :, :], wl_sb[:, :, :])
        dots = singles.tile([128, MT], F32)
        nc.vector.tensor_reduce(dots[:, :], prod[:, :, :], axis=AX.X, op=ALU.add)
        nc.vector.tensor_mul(wl_sb[:, :, :], wl_sb[:, :, :], wl_sb[:, :, :])
        nwl = singles.tile([128, MT], F32)
        nc.vector.tensor_reduce(nwl[:, :], wl_sb[:, :, :], axis=AX.X, op=ALU.add)
        lt2 = singles.tile([128, MT], F32)
        nc.scalar.activation(lt2[:, :], nwl[:, :], AF.Ln)
        rwl = singles.tile([128, MT], F32)
        nc.scalar.activation(rwl[:, :], lt2[:, :], AF.Exp, scale=-0.5)
        cl = singles.tile([128, MT], F32)
        nc.vector.tensor_mul(cl[:, :], dots[:, :], rwl[:, :])
        nc.vector.tensor_mul(cl[:, :], cl[:, :], rx_raw[:, :])
        scl = singles.tile([128, MT], F32)
        nc.vector.tensor_scalar_mul(scl[:, :], cl[:, :], S)
        e_old = singles.tile([128, MT], F32)
        nc.scalar.activation(e_old[:, :], scl[:, :], AF.Exp, bias=cb_m64[:, :])
        e_new = singles.tile([128, MT], F32)
        nc.scalar.activation(e_new[:, :], scl[:, :], AF.Exp,
                             bias=cb_m86[:, :])
        dneg = singles.tile([128, MT], F32)
        nc.vector.tensor_sub(dneg[:, :], e_old[:, :], e_new[:, :])

        # ---- reduce partials + AllReduce ----
        se_part = singles.tile([128, MT], F32)
        nc.vector.tensor_reduce(se_part[:, :], sums[:, :, :], axis=AX.X, op=ALU.add)
        ar_in = dram.tile([128, MT], F32)
        ar_out = dram.tile([128, MT], F32, addr_space="Shared")
        nc.sync.dma_start(out=ar_in[:, :], in_=se_part[:, :])
        nc.gpsimd.collective_compute(
            "AllReduce", ALU.add,
            replica_groups=[list(range(NCORES))],
            ins=[ar_in.opt()], outs=[ar_out.opt()])
        full_se = singles.tile([128, MT], F32)
        nc.sync.dma_start(out=full_se[:, :], in_=ar_out[:, :])

        # ---- logz and loss ----
        adj = singles.tile([128, MT], F32)
        nc.vector.tensor_sub(adj[:, :], full_se[:, :], dneg[:, :])
        ln_adj = singles.tile([128, MT], F32)
        # ACT Ln is inaccurate for tiny args (~1e-21); prescale into [0.1, 10]
        # via the free affine input (ln(adj*2^66) = ln(adj) + 66*ln2, the
        # constant is folded into the final bias below).
        nc.scalar.activation(ln_adj[:, :], adj[:, :], AF.Ln, scale=float(2.0 ** 66))
        if dbg is not None:
            nc.sync.dma_start(out=dbg["se"].ap()[:, :], in_=se_part[:, :])
            nc.sync.dma_start(out=dbg["adj"].ap()[:, :], in_=adj[:, :])
            nc.sync.dma_start(out=dbg["lnadj"].ap()[:, :], in_=ln_adj[:, :])
            nc.sync.dma_start(out=dbg["scl"].ap()[:, :], in_=scl[:, :])
            nc.sync.dma_start(out=dbg["dneg"].ap()[:, :], in_=dneg[:, :])
        lossv = singles.tile([128, MT], F32)
        nc.vector.tensor_sub(lossv[:, :], ln_adj[:, :], scl[:, :])
        rowsum = singles.tile([128, 1], F32)
        junk2 = singles.tile([128, MT], F32)
        nc.scalar.activation(junk2[:, :], lossv[:, :], AF.Identity,
                             accum_out=rowsum[:, :])
        # partition-axis reduce via a DRAM bounce: [128,1] -> [1,128]
        dscr = dram.tile([128, 1], F32)
        nc.sync.dma_start(out=dscr[:, :], in_=rowsum[:, :])
        row = singles.tile([1, 128], F32)
        nc.sync.dma_start(out=row[:, :], in_=dscr[:, :].rearrange("p o -> o p"))
        fin0 = singles.tile([1, 1], F32)
        junk3 = singles.tile([1, 128], F32)
        nc.scalar.activation(junk3[:, :], row[:, :], AF.Identity,
                             scale=1.0 / B, accum_out=fin0[:, :])
        fin = singles.tile([1, 1], F32)
        nc.scalar.activation(fin[:, :], fin0[:, :], AF.Identity,
                             bias=cb_p86[:1, :])
        nc.sync.dma_start(out=loss.ap()[:, :], in_=fin[:, :])


def _build():
    nc = bacc.Bacc("TRN2", target_bir_lowering=False, debug=False,
                   num_devices=NCORES)
    xt = nc.dram_tensor("xt", [D, B], F32R, kind="ExternalInput")
    xn = nc.dram_tensor("xn", [B, D], F32, kind="ExternalInput")
    wt = nc.dram_tensor("wt", [D, CLOC], F32R, kind="ExternalInput")
    wl = nc.dram_tensor("wl", [B, D], F32, kind="ExternalInput")
    loss = nc.dram_tensor("loss", [1, 1], F32, kind="ExternalOutput")
    dbg = None
    if DEBUG_DUMPS:
        dbg = {k: nc.dram_tensor("dbg_" + k, [128, MT], F32, kind="ExternalOutput")
               for k in ("se", "adj", "lnadj", "scl", "dneg")}
    with tile.TileContext(nc) as tc:
        _body(nc, tc, xt, xn, wt, wl, loss, dbg)
    nc.compile()
    return nc


def _get_nc():
    global _NC
    if _NC is None:
        _NC = _build()
    return _NC


def _prep(inputs):
    x = np.ascontiguousarray(np.asarray(inputs["input"], dtype=np.float32))
    label = np.asarray(inputs["label"]).astype(np.int64)
    w = np.asarray(inputs["weight"], dtype=np.float32)
    norms = np.sqrt((w * w).sum(axis=1, keepdims=True, dtype=np.float32))
    wn = w / np.maximum(norms, 1e-12)
    xt = np.ascontiguousarray(x.T)
    wl = np.ascontiguousarray(w[label])
    in_maps = []
    for k in range(NCORES):
        shard = np.zeros((D, CLOC), dtype=np.float32)
        shard[:, :CSHARD] = wn[k * CSHARD:(k + 1) * CSHARD].T
        in_maps.append({"xt": xt, "xn": x, "wt": shard, "wl": wl})
    return in_maps


def kernel(**inputs):
    global LAST_RESULTS
    nc = _get_nc()
    in_maps = _prep(inputs)
    res = run_bass_kernel_spmd(nc, in_maps, core_ids=list(range(NCORES)))
    LAST_RESULTS = res
    return np.asarray(res.results[0]["loss"][0, 0], dtype=np.float32)
